# revision 61
# baseline (speedup 1.0000x reference)
"""Trainium2 Bass kernel: CausalGatedD2Attention (B=1, L=2048, D=768, H=12, DH=64).

Sharding over 8 NeuronCores: 4 head-groups (3 heads each) x 2 sequence-halves
(1024 rows). Chunked causal linear attention; kv-state handoff between the two
sequence halves via a pair AllGather; output projection partial sums combined
with a 2-stage ReduceScatter over each sequence-half's 4 cores.

Self-contained: hardcodes all shapes; builds per-core shards host-side.
"""
import numpy as np

import concourse.bass as bass
from concourse import bacc
import concourse.mybir as mybir
import concourse.tile as tile
from concourse.bass_utils import run_bass_kernel_spmd
from concourse.masks import make_identity, make_upper_triangular

F32 = mybir.dt.float32
BF16 = mybir.dt.bfloat16
AF = mybir.ActivationFunctionType
ALU = mybir.AluOpType

B, L, D = 1, 2048, 768
H, DH = 12, 64
LN_EPS, DEN_EPS = 1e-5, 1e-6
SG, HG, HL = 2, 4, 3          # seq groups x head groups, heads per core
LLOC = L // SG                # 1024 rows per core
NLT = LLOC // 128             # 8 l-tiles (= attention chunks)
NDT = D // 128                # 6 contraction tiles
DLOC = HL * DH                # 192 local head columns
RG_PAIR = [[0, 4], [1, 5], [2, 6], [3, 7]]
RG_QUAD = [[0, 1, 2, 3], [4, 5, 6, 7]]

# (offset, size) n-tiles for a 192-col group and the 512-col qk pack
NT_192 = [(0, 128), (128, 64)]
# qk pack: [k(192) Z(64) q(192) Z(64)] -> active slices per 128-tile
QK_TILES = [(0, 128), (1, 64), (2, 128), (3, 64)]  # (tile idx, active rows)
K_HEAD = [(0, 0), (0, 64), (1, 0)]   # (qk tile, partition base) per local head
Q_HEAD = [(2, 0), (2, 64), (3, 0)]


def _bcast(dram_ap, p):
    return bass.AP(tensor=dram_ap.tensor, offset=dram_ap.offset,
                   ap=[[0, p]] + list(dram_ap.ap))


def _fbcast(ap, n):
    """Broadcast a [..., 1] AP along a new step-0 free dim of size n."""
    return bass.AP(tensor=ap.tensor, offset=ap.offset,
                   ap=list(ap.ap)[:-1] + [[0, n]])


def build(stage=99):
    import os
    nc = bacc.Bacc()
    x_p = nc.declare_dram_parameter("x", [LLOC, D], F32, isOutput=False)
    wqk_p = nc.declare_dram_parameter("w_qk", [D, 384], BF16, isOutput=False)
    wg4_p = nc.declare_dram_parameter("w_g4", [D, 4 * DLOC], BF16, isOutput=False)
    wpra_p = nc.declare_dram_parameter("w_pra", [128, D], BF16, isOutput=False)
    wprb_p = nc.declare_dram_parameter("w_prb", [65, D], BF16, isOutput=False)
    ncs_p = nc.declare_dram_parameter("ncs_qk", [512], F32, isOutput=False)
    csv_p = nc.declare_dram_parameter("cs_v", [DLOC], F32, isOutput=False)
    phb_p = nc.declare_dram_parameter("ph_b", [DLOC], F32, isOutput=False)
    bsa_p = nc.declare_dram_parameter("bs_a", [DLOC], F32, isOutput=False)
    bca_p = nc.declare_dram_parameter("bc_a", [DLOC], F32, isOutput=False)
    flag_p = nc.declare_dram_parameter("flag", [1], F32, isOutput=False)
    out_p = nc.declare_dram_parameter("out", [256, D], F32, isOutput=True)

    with tile.TileContext(nc) as tc, \
            tc.tile_pool(name="consts", bufs=1) as consts, \
            tc.tile_pool(name="wpool", bufs=1) as wpool, \
            tc.tile_pool(name="stats", bufs=2) as statsp, \
            tc.tile_pool(name="big", bufs=1) as big, \
            tc.tile_pool(name="work", bufs=3) as work, \
            tc.tile_pool(name="attnp", bufs=2) as attnp, \
            tc.tile_pool(name="psum", bufs=2, space="PSUM") as psp, \
            tc.tile_pool(name="dram", bufs=1, space="DRAM") as dramp:

        # ---------------- constants ----------------
        ident = consts.tile([128, 128], F32)
        make_identity(nc, ident)
        identb = consts.tile([128, 128], BF16)
        make_identity(nc, identb)
        mask3 = consts.tile([128, HL, 128], F32)   # [c2,h,c1]=1 iff c2<=c1
        for h in range(HL):
            make_upper_triangular(nc, mask3[:, h, :], val=1.0, diag=True)
        eps_t = consts.tile([128, 1], F32)
        nc.vector.memset(eps_t, LN_EPS)
        # warm-up transpose (PE observes the gpsimd identity sem once)
        wmb = psp.tile([128, 128], BF16, tag="tr")
        nc.tensor.transpose(wmb, identb, identb)

        # ---------------- x in, LN stats, batched transposes ----------------
        rstd_d = dramp.tile([LLOC], F32)
        rm_d = dramp.tile([LLOC], F32)
        rsall = statsp.tile([128, NLT], F32, tag="rsall", bufs=1)
        rmall = statsp.tile([128, NLT], F32, tag="rmall", bufs=1)
        xT = [big.tile([128, NDT, 512], BF16, name=f"xT{i}") for i in range(2)]
        xh = [big.tile([128, 4, D], F32, name=f"xh{i}") for i in range(2)]
        for lci in range(2):
            nc.sync.dma_start(out=xh[lci], in_=x_p[lci * 512:(lci + 1) * 512, :]
                              .rearrange("(t p) d -> p t d", p=128))
        flag_t = consts.tile([128, 1], F32)
        nc.sync.dma_start(out=flag_t, in_=_bcast(flag_p[:], 128))
        ncs_t = consts.tile([128, 4], F32)
        nc.sync.dma_start(out=ncs_t, in_=ncs_p[:].rearrange("(t p) -> p t", p=128))
        csvb = consts.tile([128, HL, DH], F32)
        nc.sync.dma_start(out=csvb, in_=_bcast(csv_p[:], 128).rearrange(
            "p (h e) -> p h e", h=HL))
        phb_t = consts.tile([128, 2], F32)
        bsa_t = consts.tile([128, 2], F32)
        bca_t = consts.tile([128, 2], F32)
        for t_, p_ in ((phb_t, phb_p), (bsa_t, bsa_p), (bca_t, bca_p)):
            nc.sync.dma_start(out=t_[:, 0:1], in_=p_[0:128])
            nc.sync.dma_start(out=t_[0:64, 1:2], in_=p_[128:192])
        # weights after x so the LN/transpose pipeline starts immediately
        wqk = wpool.tile([128, NDT, 384], BF16)
        nc.sync.dma_start(out=wqk, in_=wqk_p[:].rearrange("(t p) n -> p t n", p=128))
        wsa = wpool.tile([128, NDT, DLOC], BF16)
        nc.sync.dma_start(out=wsa, in_=wg4_p[:, 0:DLOC].rearrange(
            "(t p) n -> p t n", p=128))
        wca = wpool.tile([128, NDT, DLOC], BF16)
        nc.sync.dma_start(out=wca, in_=wg4_p[:, DLOC:2 * DLOC].rearrange(
            "(t p) n -> p t n", p=128))
        wpd = wpool.tile([128, NDT, DLOC], BF16)
        nc.sync.dma_start(out=wpd, in_=wg4_p[:, 2 * DLOC:3 * DLOC].rearrange(
            "(t p) n -> p t n", p=128))
        wv = wpool.tile([128, NDT, DLOC], BF16)
        nc.sync.dma_start(out=wv, in_=wg4_p[:, 3 * DLOC:4 * DLOC].rearrange(
            "(t p) n -> p t n", p=128))
        wpra = wpool.tile([128, D], BF16)
        nc.sync.dma_start(out=wpra, in_=wpra_p[:])
        wprb = wpool.tile([65, D], BF16)
        nc.sync.dma_start(out=wprb, in_=wprb_p[:])
        mvall = statsp.tile([128, NLT, 2], F32, tag="mvall", bufs=1)
        for lc in range(2):
            for i in range(4):
                st = statsp.tile([128, 2, 6], F32, tag="bst")
                for s2 in range(2):
                    nc.vector.bn_stats(out=st[:, s2, :],
                                       in_=xh[lc][:, i, s2 * 384:(s2 + 1) * 384])
                nc.vector.bn_aggr(out=mvall[:, lc * 4 + i, :], in_=st)
            # transpose to [d, l] bf16: 4 l-tiles per psum bank, one copy each
            for dt in range(NDT):
                tp = psp.tile([128, 512], F32, tag="tr")
                for i in range(4):
                    nc.tensor.transpose(tp[:, i * 128:(i + 1) * 128],
                                        xh[lc][:, i, dt * 128:(dt + 1) * 128], ident)
                if dt % 2 == 0:
                    nc.vector.tensor_copy(out=xT[lc][:, dt, :], in_=tp)
                else:
                    nc.scalar.activation(out=xT[lc][:, dt, :], in_=tp,
                                         func=AF.Copy)
        nc.scalar.activation(out=rsall, in_=mvall[:, :, 1], func=AF.Ln,
                             bias=eps_t)
        nc.scalar.activation(out=rsall, in_=rsall, func=AF.Exp, scale=-0.5)
        nc.vector.tensor_mul(out=rmall, in0=mvall[:, :, 0], in1=rsall)
        rs_l = [rsall[:, lt:lt + 1] for lt in range(NLT)]
        rm_l = [rmall[:, lt:lt + 1] for lt in range(NLT)]

        nc.sync.dma_start(out=rstd_d[:].rearrange("(t p) -> p t", p=128), in_=rsall)
        nc.sync.dma_start(out=rm_d[:].rearrange("(t p) -> p t", p=128), in_=rmall)
        rstdb = big.tile([128, LLOC], F32)
        nc.sync.dma_start(out=rstdb, in_=_bcast(rstd_d[:], 128))
        rmb = big.tile([128, LLOC], F32)
        nc.sync.dma_start(out=rmb, in_=_bcast(rm_d[:], 128))

        # ---------------- per-half: matmuls, gate, features, local states ----
        qkpre = big.tile([128, 4, LLOC], F32)
        qkf = big.tile([128, 4, LLOC], BF16)
        vaug = [attnp.tile([128, HL, 65], BF16, name=f"vaug{c}", tag="vaug",
                           bufs=NLT) for c in range(NLT)]
        krow = [attnp.tile([128, DLOC], BF16, name=f"krow{c}", tag="krow",
                           bufs=NLT) for c in range(NLT)]
        stot = big.tile([128, 2, 65], BF16)
        nc.vector.memset(stot, 0.0)
        sloc = big.tile([128, NLT, 2, 65], BF16)
        nc.vector.memset(sloc, 0.0)
        attn_i = [attnp.tile([128, HL, 65], F32, name=f"attni{c}", tag="attni",
                             bufs=NLT) for c in range(NLT)]
        st_d = dramp.tile([128, 2, 65], BF16)
        stg_d = dramp.tile([2, 128, 2, 65], BF16)

        def mm_ii(wt, ofs, nsz, lc):
            pm = psp.tile([128, 512], F32, tag="mm", bufs=3)
            for dt in range(NDT):
                nc.tensor.matmul(pm[:nsz, :], wt[:, dt, ofs:ofs + nsz],
                                 xT[lc][:, dt, :], start=(dt == 0),
                                 stop=(dt == NDT - 1))
            return pm

        def qk_fold(nt, nsz, lc, lsl, ofs):
            pm = mm_ii(wqk, ofs, nsz, lc)
            t2m = work.tile([128, 512], F32, tag="t2m", bufs=4)
            nc.vector.tensor_mul(out=t2m[:nsz], in0=pm[:nsz], in1=rstdb[:nsz, lsl])
            nc.vector.scalar_tensor_tensor(
                out=qkpre[:nsz, nt, lsl], in0=rmb[:nsz, lsl],
                scalar=ncs_t[:nsz, nt:nt + 1], in1=t2m[:nsz],
                op0=ALU.mult, op1=ALU.add)

        def elu1(nt, nsz, lsl):
            w = lsl.stop - lsl.start
            mn = work.tile([128, 1024], F32, tag="mn", bufs=4)
            nc.scalar.activation(out=mn[:nsz, :w], in_=qkpre[:nsz, nt, lsl],
                                 func=AF.Relu, scale=-1.0)
            nc.scalar.activation(out=mn[:nsz, :w], in_=mn[:nsz, :w], func=AF.Exp,
                                 scale=-1.0)
            rl = work.tile([128, 1024], F32, tag="rl", bufs=4)
            nc.scalar.activation(out=rl[:nsz, :w], in_=qkpre[:nsz, nt, lsl],
                                 func=AF.Relu)
            eng = nc.gpsimd if w == LLOC else nc.vector
            eng.tensor_add(out=qkf[:nsz, nt, lsl], in0=mn[:nsz, :w],
                           in1=rl[:nsz, :w])

        for lc in range(2):
            lsl = slice(lc * 512, (lc + 1) * 512)
            # k tiles with LN fold (weight cols packed without Z pads)
            for nt, nsz, ofs in ((0, 128, 0), (1, 64, 128)):
                qk_fold(nt, nsz, lc, lsl, ofs)
            # gate: softplus(a)=ln(1+e^a); ACT set batching: exp/ln .. sin .. exp
            spas, cpas, csts = [], [], []
            for tt, nsz in NT_192:
                ti = tt // 128
                spa = work.tile([128, 512], F32, tag="spa", bufs=4)
                pm = mm_ii(wsa, tt, nsz, lc)
                nc.scalar.activation(out=spa[:nsz], in_=pm[:nsz], func=AF.Exp,
                                     bias=bsa_t[:nsz, ti:ti + 1])
                nc.scalar.activation(out=spa[:nsz], in_=spa[:nsz], func=AF.Ln,
                                     bias=1.0)
                cpa = work.tile([128, 512], F32, tag="cpa", bufs=4)
                pm = mm_ii(wca, tt, nsz, lc)
                nc.scalar.activation(out=cpa[:nsz], in_=pm[:nsz], func=AF.Exp,
                                     bias=bca_t[:nsz, ti:ti + 1])
                nc.scalar.activation(out=cpa[:nsz], in_=cpa[:nsz], func=AF.Ln,
                                     bias=1.0)
                spas.append(spa)
                cpas.append(cpa)
            for tt, nsz in NT_192:
                ti = tt // 128
                cst = work.tile([128, 512], F32, tag="cst", bufs=4)
                pm = mm_ii(wpd, tt, nsz, lc)
                nc.scalar.activation(out=cst[:nsz], in_=pm[:nsz], func=AF.Sin,
                                     bias=phb_t[:nsz, ti:ti + 1])
                csts.append(cst)
            # v (row layout) for this half
            for i in range(4):
                lt = lc * 4 + i
                pv = psp.tile([128, 512], F32, tag="mm", bufs=3)
                for dt in range(NDT):
                    nc.tensor.matmul(pv[:, 0:DLOC],
                                     xT[lc][:, dt, i * 128:(i + 1) * 128],
                                     wv[:, dt, :], start=(dt == 0),
                                     stop=(dt == NDT - 1))
                a1 = work.tile([128, HL, DH], F32, tag="a1")
                nc.scalar.activation(out=a1, in_=csvb, func=AF.Identity,
                                     scale=rm_l[lt])
                nc.vector.scalar_tensor_tensor(
                    out=vaug[lt][:, :, 0:64],
                    in0=pv[:, 0:DLOC].rearrange("p (h e) -> p h e", h=HL),
                    scalar=rs_l[lt], in1=a1, op0=ALU.mult, op1=ALU.subtract)
                nc.vector.memset(vaug[lt][:, :, 64:65], 1.0)
            for i, (tt, nsz) in enumerate(NT_192):
                ti = tt // 128
                spa, cpa, cst = spas[i], cpas[i], csts[i]
                nc.gpsimd.tensor_mul(out=spa[:nsz], in0=spa[:nsz], in1=cpa[:nsz])
                nc.vector.tensor_mul(out=spa[:nsz], in0=spa[:nsz], in1=cst[:nsz])
                nc.scalar.activation(out=cpa[:nsz], in_=spa[:nsz], func=AF.Exp,
                                     scale=-1.0)
                nc.vector.tensor_scalar_add(out=cpa[:nsz], in0=cpa[:nsz],
                                            scalar1=1.0)
                nc.vector.reciprocal(out=cpa[:nsz], in_=cpa[:nsz])
                nc.gpsimd.tensor_mul(out=qkpre[:nsz, ti, lsl],
                                     in0=qkpre[:nsz, ti, lsl], in1=cpa[:nsz])
            # k elu first; k row layout + chunk states as early as possible
            for nt, nsz in ((0, 128), (1, 64)):
                elu1(nt, nsz, lsl)
            for i in range(4):
                c = lc * 4 + i
                csl = slice(c * 128, (c + 1) * 128)
                tpk = psp.tile([128, 512], BF16, tag="tr")
                nc.tensor.transpose(tpk[:, 0:128], qkf[:, 0, csl], identb)
                nc.tensor.transpose(tpk[:, 128:192], qkf[0:64, 1, csl],
                                    identb[0:64, 0:64])
                nc.scalar.activation(out=krow[c][:, 0:128], in_=tpk[:, 0:128],
                                     func=AF.Copy)
                nc.vector.tensor_copy(out=krow[c][:, 128:192],
                                      in_=tpk[:, 128:192])
            # local kv chunk states for this half
            for i in range(4):
                c = lc * 4 + i
                psa = psp.tile([128, 2, 65], F32, tag="sl", bufs=1)
                nc.tensor.matmul(psa[0:64, 0, :], krow[c][:, 0:64],
                                 vaug[c][:, 0, :], start=True, stop=True)
                nc.tensor.matmul(psa[64:128, 0, :], krow[c][:, 64:128],
                                 vaug[c][:, 1, :], start=True, stop=True,
                                 tile_position=(0, 64))
                nc.tensor.matmul(psa[0:64, 1, :], krow[c][:, 128:192],
                                 vaug[c][:, 2, :], start=True, stop=True)
                nc.vector.tensor_copy(out=sloc[:, c, :, :], in_=psa)
                nc.vector.tensor_add(out=stot, in0=stot, in1=psa)


        # fire the state handoff as soon as all local chunk states exist;
        # q-side folds + intra-chunk attention fully overlap the AllGather
        nc.sync.dma_start(out=st_d[:], in_=stot)
        nc.gpsimd.collective_compute("AllGather", ALU.bypass,
                                     replica_groups=RG_PAIR,
                                     ins=[st_d[:]], outs=[stg_d[:]])
        for lc in range(2):
            lsl = slice(lc * 512, (lc + 1) * 512)
            for nt, nsz, ofs in ((2, 128, 192), (3, 64, 320)):
                qk_fold(nt, nsz, lc, lsl, ofs)
        for nt, nsz in ((2, 128), (3, 64)):
            elu1(nt, nsz, slice(0, LLOC))
        for c in range(NLT):
            csl = slice(c * 128, (c + 1) * 128)
            stm = attnp.tile([128, HL, 128], BF16, tag="stm", bufs=4)
            pst2 = psp.tile([128, 2, 128], F32, tag="tr")
            for j, h in enumerate((0, 2)):       # base-0 heads share a psum
                kt, kb = K_HEAD[h]
                qt, qb = Q_HEAD[h]
                nc.tensor.matmul(pst2[:, j, :],
                                 qkf[kb:kb + 64, kt, csl],
                                 qkf[qb:qb + 64, qt, csl],
                                 start=True, stop=True)
            nc.vector.tensor_mul(out=stm[:, 0:2, :], in0=pst2,
                                 in1=mask3[:, 0:2, :])
            pst = psp.tile([128, 128], F32, tag="tr")
            nc.tensor.matmul(pst, qkf[64:128, 0, csl], qkf[64:128, 2, csl],
                             start=True, stop=True)
            nc.vector.tensor_mul(out=stm[:, 2, :], in0=pst, in1=mask3[:, 2, :])
            po = psp.tile([128, HL, 65], F32, tag="out")
            for j, h in enumerate((0, 2, 1)):
                nc.tensor.matmul(po[:, h, :], stm[:, j, :], vaug[c][:, h, :],
                                 start=True, stop=True)
            nc.scalar.activation(out=attn_i[c], in_=po, func=AF.Copy)

        # ------- local prefix states (independent of the AllGather) -------
        sacc = big.tile([128, 2, 65], F32)
        nc.vector.memset(sacc, 0.0)
        spre = big.tile([128, NLT, 2, 65], BF16)
        for c in range(1, NLT):
            nc.vector.tensor_add(out=sacc, in0=sacc, in1=sloc[:, c - 1])
            nc.vector.tensor_copy(out=spre[:, c], in_=sacc)

        # ------------- inter-chunk attention + projection + RS -------------
        attnTA = big.tile([128, LLOC], BF16)
        attnTB = big.tile([65, LLOC], BF16)
        nc.vector.memset(attnTB[64:65, :], 1.0)
        part_d = dramp.tile([LLOC, D], F32)
        rs_d = dramp.tile([256, D], F32)
        attnr = [attnp.tile([128, DLOC], BF16, name=f"attnr{c}", tag="attnr",
                            bufs=8) for c in range(NLT)]
        # local-prefix contribution: runs inside the AllGather window
        for c in range(1, NLT):
            csl = slice(c * 128, (c + 1) * 128)
            for h in range(HL):
                qt, qb = Q_HEAD[h]
                kb = K_HEAD[h][1]
                sprh = spre[kb:kb + 64, c, 0, :] if h < 2 else spre[0:64, c, 1, :]
                po2 = psp.tile([128, 65], F32, tag="out")
                nc.tensor.matmul(po2, qkf[qb:qb + 64, qt, csl], sprh,
                                 start=True, stop=True)
                nc.vector.tensor_add(out=attn_i[c][:, h, :],
                                     in0=attn_i[c][:, h, :], in1=po2)
        # peer-state correction + epilogue (post-AllGather)
        sgsb = big.tile([128, 2, 65], BF16)
        nc.sync.dma_start(out=sgsb, in_=stg_d[0])
        sinit = big.tile([128, 2, 65], BF16)
        nc.vector.tensor_scalar_mul(out=sinit, in0=sgsb, scalar1=flag_t)
        for c in range(NLT):
            csl = slice(c * 128, (c + 1) * 128)
            for h in range(HL):
                kt, kb = K_HEAD[h]
                qt, qb = Q_HEAD[h]
                sih = sinit[kb:kb + 64, 0, :] if h < 2 else sinit[0:64, 1, :]
                po2 = psp.tile([128, 65], F32, tag="out")
                nc.tensor.matmul(po2, qkf[qb:qb + 64, qt, csl], sih,
                                 start=True, stop=True)
                nc.vector.tensor_add(out=attn_i[c][:, h, :],
                                     in0=attn_i[c][:, h, :], in1=po2)
            den = attnp.tile([128, HL], F32, tag="den", bufs=3)
            nc.gpsimd.tensor_scalar_add(out=den, in0=attn_i[c][:, :, 64],
                                        scalar1=DEN_EPS)
            nc.vector.reciprocal(out=den, in_=den)
            nc.gpsimd.tensor_mul(
                out=attnr[c].rearrange("p (h e) -> p h e", h=HL),
                in0=attn_i[c][:, :, 0:64], in1=_fbcast(den[:, :, None], 64))
            tp = psp.tile([128, 512], BF16, tag="tr")
            nc.tensor.transpose(tp[:, 0:128], attnr[c][:, 0:128], identb)
            nc.tensor.transpose(tp[0:64, 128:256], attnr[c][:, 128:192], identb)
            nc.vector.tensor_copy(out=attnTA[:, csl], in_=tp[:, 0:128])
            nc.scalar.activation(out=attnTB[0:64, csl], in_=tp[0:64, 128:256],
                                 func=AF.Copy)
            for ns in range(2):
                nsl = slice(ns * 384, (ns + 1) * 384)
                pp = psp.tile([128, 512], F32, tag="mm", bufs=3)
                nc.tensor.matmul(pp[:, 0:384], attnTA[:, csl], wpra[:, nsl],
                                 start=True, stop=False)
                nc.tensor.matmul(pp[:, 0:384], attnTB[:, csl], wprb[:, nsl],
                                 start=False, stop=True)
                pout = work.tile([128, 384], F32, tag="pout")
                nc.vector.tensor_copy(out=pout, in_=pp[:, 0:384])
                nc.sync.dma_start(out=part_d[csl, nsl], in_=pout)
        nc.gpsimd.collective_compute(
            "ReduceScatter", ALU.add, replica_groups=RG_QUAD,
            ins=[part_d[:]], outs=[rs_d[:]])
        nc.sync.dma_start(out=out_p[:], in_=rs_d[:])

    nc.finalize()
    return nc


_NC_CACHE = None


def _get_nc():
    global _NC_CACHE
    if _NC_CACHE is None:
        import os
        _NC_CACHE = build(stage=int(os.environ.get("KSTAGE", "99")))
    return _NC_CACHE


def _prep_in_maps(inputs):
    x = np.ascontiguousarray(np.asarray(inputs["x"], np.float32))[0]
    W_qkv = np.asarray(inputs["W_qkv"], np.float32)
    b_qkv = np.asarray(inputs["b_qkv"], np.float32)
    W_sem = np.asarray(inputs["W_sem"], np.float32)
    b_sem = np.asarray(inputs["b_sem"], np.float32)
    W_ctx = np.asarray(inputs["W_ctx"], np.float32)
    b_ctx = np.asarray(inputs["b_ctx"], np.float32)
    W_proj = np.asarray(inputs["W_proj"], np.float32)
    b_proj = np.asarray(inputs["b_proj"], np.float32)
    ln_g = np.asarray(inputs["ln_g"], np.float32)
    ln_b = np.asarray(inputs["ln_b"], np.float32)

    Wg = ln_g[:, None] * W_qkv
    bias2 = ln_b @ W_qkv + b_qkv
    assert not np.any(bias2), "nonzero qkv bias not supported by this kernel"

    import ml_dtypes
    c = lambda a: np.ascontiguousarray(a, dtype=np.float32)
    cb = lambda a: np.ascontiguousarray(np.asarray(a, np.float32).astype(ml_dtypes.bfloat16))
    in_maps = []
    for core in range(8):
        sg, hg = core // 4, core % 4
        idx = slice(DLOC * hg, DLOC * hg + DLOC)
        Wk_c = Wg[:, 768:1536][:, idx]
        Wq_c = Wg[:, 0:768][:, idx]
        Z = np.zeros((D, 64), np.float32)
        w_qk = np.concatenate([Wk_c, Wq_c], 1)          # [768, 384], no pads
        w_qk_pad = np.concatenate([Wk_c, Z, Wq_c, Z], 1)  # ncs keeps tile layout
        Wv_c = Wg[:, 1536:][:, idx]
        in_maps.append({
            "x": c(x[LLOC * sg:LLOC * sg + LLOC]),
            "w_qk": cb(w_qk),
            "w_g4": cb(np.concatenate([
                W_sem[:, 0:768][:, idx], W_ctx[:, 0:768][:, idx],
                W_sem[:, 768:][:, idx] - W_ctx[:, 768:][:, idx], Wv_c], 1)),
            "w_pra": cb(W_proj[idx, :][0:128]),
            "w_prb": cb(np.concatenate([W_proj[idx, :][128:192],
                                        b_proj[None, :] / HG], 0)),
            "ncs_qk": c(-w_qk_pad.sum(0)),
            "cs_v": c(Wv_c.sum(0)),
            "ph_b": c(np.pi / 2 + (b_sem[768:][idx] - b_ctx[768:][idx])),
            "bs_a": c(b_sem[:768][idx]),
            "bc_a": c(b_ctx[:768][idx]),
            "flag": c(np.array([float(sg)])),
        })
    return in_maps


def _run(inputs, trace=False):
    nc = _get_nc()
    in_maps = _prep_in_maps(inputs)
    res = run_bass_kernel_spmd(nc, in_maps, core_ids=list(range(8)), trace=trace)
    out = np.zeros((L, D), np.float32)
    for core in range(8):
        sg, hg = core // 4, core % 4
        o = res.results[core]["out"]          # [256, D]
        r0 = LLOC * sg + 256 * hg
        out[r0:r0 + 256] = o
    return out[None], res


def kernel(**inputs):
    out, _ = _run(inputs, trace=False)
    return out


# revision 63
# speedup vs baseline: 1.0024x; 1.0024x over previous
"""Trainium2 Bass kernel: CausalGatedD2Attention (B=1, L=2048, D=768, H=12, DH=64).

Sharding over 8 NeuronCores: 4 head-groups (3 heads each) x 2 sequence-halves
(1024 rows). Chunked causal linear attention; kv-state handoff between the two
sequence halves via a pair AllGather; output projection partial sums combined
with a 2-stage ReduceScatter over each sequence-half's 4 cores.

Self-contained: hardcodes all shapes; builds per-core shards host-side.
"""
import numpy as np

import concourse.bass as bass
from concourse import bacc
import concourse.mybir as mybir
import concourse.tile as tile
from concourse.bass_utils import run_bass_kernel_spmd
from concourse.masks import make_identity, make_upper_triangular

F32 = mybir.dt.float32
BF16 = mybir.dt.bfloat16
AF = mybir.ActivationFunctionType
ALU = mybir.AluOpType

B, L, D = 1, 2048, 768
H, DH = 12, 64
LN_EPS, DEN_EPS = 1e-5, 1e-6
SG, HG, HL = 2, 4, 3          # seq groups x head groups, heads per core
LLOC = L // SG                # 1024 rows per core
NLT = LLOC // 128             # 8 l-tiles (= attention chunks)
NDT = D // 128                # 6 contraction tiles
DLOC = HL * DH                # 192 local head columns
RG_PAIR = [[0, 4], [1, 5], [2, 6], [3, 7]]
RG_QUAD = [[0, 1, 2, 3], [4, 5, 6, 7]]

# (offset, size) n-tiles for a 192-col group and the 512-col qk pack
NT_192 = [(0, 128), (128, 64)]
# qk pack: [k(192) Z(64) q(192) Z(64)] -> active slices per 128-tile
QK_TILES = [(0, 128), (1, 64), (2, 128), (3, 64)]  # (tile idx, active rows)
K_HEAD = [(0, 0), (0, 64), (1, 0)]   # (qk tile, partition base) per local head
Q_HEAD = [(2, 0), (2, 64), (3, 0)]


def _bcast(dram_ap, p):
    return bass.AP(tensor=dram_ap.tensor, offset=dram_ap.offset,
                   ap=[[0, p]] + list(dram_ap.ap))


def _fbcast(ap, n):
    """Broadcast a [..., 1] AP along a new step-0 free dim of size n."""
    return bass.AP(tensor=ap.tensor, offset=ap.offset,
                   ap=list(ap.ap)[:-1] + [[0, n]])


def build(stage=99):
    import os
    nc = bacc.Bacc()
    x_p = nc.declare_dram_parameter("x", [LLOC, D], F32, isOutput=False)
    wqk_p = nc.declare_dram_parameter("w_qk", [D, 384], BF16, isOutput=False)
    wg4_p = nc.declare_dram_parameter("w_g4", [D, 4 * DLOC], BF16, isOutput=False)
    wpra_p = nc.declare_dram_parameter("w_pra", [128, D], BF16, isOutput=False)
    wprb_p = nc.declare_dram_parameter("w_prb", [65, D], BF16, isOutput=False)
    ncs_p = nc.declare_dram_parameter("ncs_qk", [512], F32, isOutput=False)
    csv_p = nc.declare_dram_parameter("cs_v", [DLOC], F32, isOutput=False)
    phb_p = nc.declare_dram_parameter("ph_b", [DLOC], F32, isOutput=False)
    bsa_p = nc.declare_dram_parameter("bs_a", [DLOC], F32, isOutput=False)
    bca_p = nc.declare_dram_parameter("bc_a", [DLOC], F32, isOutput=False)
    flag_p = nc.declare_dram_parameter("flag", [1], F32, isOutput=False)
    out_p = nc.declare_dram_parameter("out", [256, D], F32, isOutput=True)

    with tile.TileContext(nc) as tc, \
            tc.tile_pool(name="consts", bufs=1) as consts, \
            tc.tile_pool(name="wpool", bufs=1) as wpool, \
            tc.tile_pool(name="stats", bufs=2) as statsp, \
            tc.tile_pool(name="big", bufs=1) as big, \
            tc.tile_pool(name="work", bufs=3) as work, \
            tc.tile_pool(name="attnp", bufs=2) as attnp, \
            tc.tile_pool(name="psum", bufs=2, space="PSUM") as psp, \
            tc.tile_pool(name="dram", bufs=1, space="DRAM") as dramp:

        # ---------------- constants ----------------
        ident = consts.tile([128, 128], F32)
        make_identity(nc, ident)
        identb = consts.tile([128, 128], BF16)
        make_identity(nc, identb)
        mask3 = consts.tile([128, HL, 128], F32)   # [c2,h,c1]=1 iff c2<=c1
        for h in range(HL):
            make_upper_triangular(nc, mask3[:, h, :], val=1.0, diag=True)
        eps_t = consts.tile([128, 1], F32)
        nc.vector.memset(eps_t, LN_EPS)
        # warm-up transpose (PE observes the gpsimd identity sem once)
        wmb = psp.tile([128, 128], BF16, tag="tr")
        nc.tensor.transpose(wmb, identb, identb)

        # ---------------- x in, LN stats, batched transposes ----------------
        rstd_d = dramp.tile([LLOC], F32)
        rm_d = dramp.tile([LLOC], F32)
        rsall = statsp.tile([128, NLT], F32, tag="rsall", bufs=1)
        rmall = statsp.tile([128, NLT], F32, tag="rmall", bufs=1)
        xT = [big.tile([128, NDT, 512], BF16, name=f"xT{i}") for i in range(2)]
        xh = [big.tile([128, 4, D], F32, name=f"xh{i}") for i in range(2)]
        for lci in range(2):
            nc.sync.dma_start(out=xh[lci], in_=x_p[lci * 512:(lci + 1) * 512, :]
                              .rearrange("(t p) d -> p t d", p=128))
        flag_t = consts.tile([128, 1], F32)
        nc.sync.dma_start(out=flag_t, in_=_bcast(flag_p[:], 128))
        ncs_t = consts.tile([128, 4], F32)
        nc.sync.dma_start(out=ncs_t, in_=ncs_p[:].rearrange("(t p) -> p t", p=128))
        csvb = consts.tile([128, HL, DH], F32)
        nc.sync.dma_start(out=csvb, in_=_bcast(csv_p[:], 128).rearrange(
            "p (h e) -> p h e", h=HL))
        phb_t = consts.tile([128, 2], F32)
        bsa_t = consts.tile([128, 2], F32)
        bca_t = consts.tile([128, 2], F32)
        for t_, p_ in ((phb_t, phb_p), (bsa_t, bsa_p), (bca_t, bca_p)):
            nc.sync.dma_start(out=t_[:, 0:1], in_=p_[0:128])
            nc.sync.dma_start(out=t_[0:64, 1:2], in_=p_[128:192])
        # weights after x so the LN/transpose pipeline starts immediately
        wqk = wpool.tile([128, NDT, 384], BF16)
        nc.sync.dma_start(out=wqk, in_=wqk_p[:].rearrange("(t p) n -> p t n", p=128))
        wsa = wpool.tile([128, NDT, DLOC], BF16)
        nc.sync.dma_start(out=wsa, in_=wg4_p[:, 0:DLOC].rearrange(
            "(t p) n -> p t n", p=128))
        wca = wpool.tile([128, NDT, DLOC], BF16)
        nc.sync.dma_start(out=wca, in_=wg4_p[:, DLOC:2 * DLOC].rearrange(
            "(t p) n -> p t n", p=128))
        wpd = wpool.tile([128, NDT, DLOC], BF16)
        nc.sync.dma_start(out=wpd, in_=wg4_p[:, 2 * DLOC:3 * DLOC].rearrange(
            "(t p) n -> p t n", p=128))
        wv = wpool.tile([128, NDT, DLOC], BF16)
        nc.sync.dma_start(out=wv, in_=wg4_p[:, 3 * DLOC:4 * DLOC].rearrange(
            "(t p) n -> p t n", p=128))
        wpra = wpool.tile([128, D], BF16)
        nc.sync.dma_start(out=wpra, in_=wpra_p[:])
        wprb = wpool.tile([65, D], BF16)
        nc.sync.dma_start(out=wprb, in_=wprb_p[:])
        mvall = statsp.tile([128, NLT, 2], F32, tag="mvall", bufs=1)
        for lc in range(2):
            for i in range(4):
                st = statsp.tile([128, 2, 6], F32, tag="bst")
                for s2 in range(2):
                    nc.vector.bn_stats(out=st[:, s2, :],
                                       in_=xh[lc][:, i, s2 * 384:(s2 + 1) * 384])
                nc.vector.bn_aggr(out=mvall[:, lc * 4 + i, :], in_=st)
            # transpose to [d, l] bf16: 4 l-tiles per psum bank, one copy each
            for dt in range(NDT):
                tp = psp.tile([128, 512], F32, tag="tr")
                for i in range(4):
                    nc.tensor.transpose(tp[:, i * 128:(i + 1) * 128],
                                        xh[lc][:, i, dt * 128:(dt + 1) * 128], ident)
                if dt % 2 == 0:
                    nc.vector.tensor_copy(out=xT[lc][:, dt, :], in_=tp)
                else:
                    nc.scalar.activation(out=xT[lc][:, dt, :], in_=tp,
                                         func=AF.Copy)
        nc.scalar.activation(out=rsall, in_=mvall[:, :, 1], func=AF.Ln,
                             bias=eps_t)
        nc.scalar.activation(out=rsall, in_=rsall, func=AF.Exp, scale=-0.5)
        nc.vector.tensor_mul(out=rmall, in0=mvall[:, :, 0], in1=rsall)
        rs_l = [rsall[:, lt:lt + 1] for lt in range(NLT)]
        rm_l = [rmall[:, lt:lt + 1] for lt in range(NLT)]

        nc.sync.dma_start(out=rstd_d[:].rearrange("(t p) -> p t", p=128), in_=rsall)
        nc.sync.dma_start(out=rm_d[:].rearrange("(t p) -> p t", p=128), in_=rmall)
        rstdb = big.tile([128, LLOC], F32)
        nc.sync.dma_start(out=rstdb, in_=_bcast(rstd_d[:], 128))
        rmb = big.tile([128, LLOC], F32)
        nc.sync.dma_start(out=rmb, in_=_bcast(rm_d[:], 128))

        # ---------------- per-half: matmuls, gate, features, local states ----
        qkpre = big.tile([128, 4, LLOC], F32)
        qkf = big.tile([128, 4, LLOC], BF16)
        vaug = [attnp.tile([128, HL, 65], BF16, name=f"vaug{c}", tag="vaug",
                           bufs=NLT) for c in range(NLT)]
        krow = [attnp.tile([128, DLOC], BF16, name=f"krow{c}", tag="krow",
                           bufs=NLT) for c in range(NLT)]
        stot = big.tile([128, 2, 65], BF16)
        nc.vector.memset(stot, 0.0)
        sloc = big.tile([128, NLT, 2, 65], BF16)
        nc.vector.memset(sloc, 0.0)
        attn_i = [attnp.tile([128, HL, 65], F32, name=f"attni{c}", tag="attni",
                             bufs=NLT) for c in range(NLT)]
        st_d = dramp.tile([128, 2, 65], BF16)
        stg_d = dramp.tile([2, 128, 2, 65], BF16)

        def mm_ii(wt, ofs, nsz, lc):
            pm = psp.tile([128, 512], F32, tag="mm", bufs=3)
            for dt in range(NDT):
                nc.tensor.matmul(pm[:nsz, :], wt[:, dt, ofs:ofs + nsz],
                                 xT[lc][:, dt, :], start=(dt == 0),
                                 stop=(dt == NDT - 1))
            return pm

        def qk_fold(nt, nsz, lc, lsl, ofs):
            pm = mm_ii(wqk, ofs, nsz, lc)
            t2m = work.tile([128, 512], F32, tag="t2m", bufs=4)
            nc.vector.tensor_mul(out=t2m[:nsz], in0=pm[:nsz], in1=rstdb[:nsz, lsl])
            nc.vector.scalar_tensor_tensor(
                out=qkpre[:nsz, nt, lsl], in0=rmb[:nsz, lsl],
                scalar=ncs_t[:nsz, nt:nt + 1], in1=t2m[:nsz],
                op0=ALU.mult, op1=ALU.add)

        def elu1(nt, nsz, lsl):
            w = lsl.stop - lsl.start
            mn = work.tile([128, 1024], F32, tag="mn", bufs=4)
            nc.scalar.activation(out=mn[:nsz, :w], in_=qkpre[:nsz, nt, lsl],
                                 func=AF.Relu, scale=-1.0)
            nc.scalar.activation(out=mn[:nsz, :w], in_=mn[:nsz, :w], func=AF.Exp,
                                 scale=-1.0)
            rl = work.tile([128, 1024], F32, tag="rl", bufs=4)
            nc.scalar.activation(out=rl[:nsz, :w], in_=qkpre[:nsz, nt, lsl],
                                 func=AF.Relu)
            eng = nc.gpsimd if w == LLOC else nc.vector
            eng.tensor_add(out=qkf[:nsz, nt, lsl], in0=mn[:nsz, :w],
                           in1=rl[:nsz, :w])

        for lc in range(2):
            lsl = slice(lc * 512, (lc + 1) * 512)
            # k tiles with LN fold (weight cols packed without Z pads)
            for nt, nsz, ofs in ((0, 128, 0), (1, 64, 128)):
                qk_fold(nt, nsz, lc, lsl, ofs)
            # gate: softplus(a)=ln(1+e^a); ACT set batching: exp/ln .. sin .. exp
            spas, cpas, csts = [], [], []
            for tt, nsz in NT_192:
                ti = tt // 128
                spa = work.tile([128, 512], F32, tag="spa", bufs=4)
                pm = mm_ii(wsa, tt, nsz, lc)
                nc.scalar.activation(out=spa[:nsz], in_=pm[:nsz], func=AF.Exp,
                                     bias=bsa_t[:nsz, ti:ti + 1])
                nc.scalar.activation(out=spa[:nsz], in_=spa[:nsz], func=AF.Ln,
                                     bias=1.0)
                cpa = work.tile([128, 512], F32, tag="cpa", bufs=4)
                pm = mm_ii(wca, tt, nsz, lc)
                nc.scalar.activation(out=cpa[:nsz], in_=pm[:nsz], func=AF.Exp,
                                     bias=bca_t[:nsz, ti:ti + 1])
                nc.scalar.activation(out=cpa[:nsz], in_=cpa[:nsz], func=AF.Ln,
                                     bias=1.0)
                spas.append(spa)
                cpas.append(cpa)
            for tt, nsz in NT_192:
                ti = tt // 128
                cst = work.tile([128, 512], F32, tag="cst", bufs=4)
                pm = mm_ii(wpd, tt, nsz, lc)
                nc.scalar.activation(out=cst[:nsz], in_=pm[:nsz], func=AF.Sin,
                                     bias=phb_t[:nsz, ti:ti + 1])
                csts.append(cst)
            # v (row layout) for this half
            for i in range(4):
                lt = lc * 4 + i
                pv = psp.tile([128, 512], F32, tag="mm", bufs=3)
                for dt in range(NDT):
                    nc.tensor.matmul(pv[:, 0:DLOC],
                                     xT[lc][:, dt, i * 128:(i + 1) * 128],
                                     wv[:, dt, :], start=(dt == 0),
                                     stop=(dt == NDT - 1))
                a1 = work.tile([128, HL, DH], F32, tag="a1")
                nc.scalar.activation(out=a1, in_=csvb, func=AF.Identity,
                                     scale=rm_l[lt])
                nc.vector.scalar_tensor_tensor(
                    out=vaug[lt][:, :, 0:64],
                    in0=pv[:, 0:DLOC].rearrange("p (h e) -> p h e", h=HL),
                    scalar=rs_l[lt], in1=a1, op0=ALU.mult, op1=ALU.subtract)
                nc.vector.memset(vaug[lt][:, :, 64:65], 1.0)
            for i, (tt, nsz) in enumerate(NT_192):
                ti = tt // 128
                spa, cpa, cst = spas[i], cpas[i], csts[i]
                nc.gpsimd.tensor_mul(out=spa[:nsz], in0=spa[:nsz], in1=cpa[:nsz])
                nc.vector.tensor_mul(out=spa[:nsz], in0=spa[:nsz], in1=cst[:nsz])
                nc.scalar.activation(out=cpa[:nsz], in_=spa[:nsz], func=AF.Exp,
                                     scale=-1.0)
                nc.vector.tensor_scalar_add(out=cpa[:nsz], in0=cpa[:nsz],
                                            scalar1=1.0)
                nc.vector.reciprocal(out=cpa[:nsz], in_=cpa[:nsz])
                nc.gpsimd.tensor_mul(out=qkpre[:nsz, ti, lsl],
                                     in0=qkpre[:nsz, ti, lsl], in1=cpa[:nsz])
            # k elu first; k row layout + chunk states as early as possible
            for nt, nsz in ((0, 128), (1, 64)):
                elu1(nt, nsz, lsl)
            for i in range(4):
                c = lc * 4 + i
                csl = slice(c * 128, (c + 1) * 128)
                tpk = psp.tile([128, 512], BF16, tag="tr")
                nc.tensor.transpose(tpk[:, 0:128], qkf[:, 0, csl], identb)
                nc.tensor.transpose(tpk[:, 128:192], qkf[0:64, 1, csl],
                                    identb[0:64, 0:64])
                nc.scalar.activation(out=krow[c][:, 0:128], in_=tpk[:, 0:128],
                                     func=AF.Copy)
                nc.vector.tensor_copy(out=krow[c][:, 128:192],
                                      in_=tpk[:, 128:192])
            # local kv chunk states for this half
            for i in range(4):
                c = lc * 4 + i
                psa = psp.tile([128, 2, 65], F32, tag="sl", bufs=1)
                nc.tensor.matmul(psa[0:64, 0, :], krow[c][:, 0:64],
                                 vaug[c][:, 0, :], start=True, stop=True)
                nc.tensor.matmul(psa[64:128, 0, :], krow[c][:, 64:128],
                                 vaug[c][:, 1, :], start=True, stop=True,
                                 tile_position=(0, 64))
                nc.tensor.matmul(psa[0:64, 1, :], krow[c][:, 128:192],
                                 vaug[c][:, 2, :], start=True, stop=True)
                nc.vector.tensor_copy(out=sloc[:, c, :, :], in_=psa)
                nc.vector.tensor_add(out=stot, in0=stot, in1=psa)


        # fire the state handoff as soon as all local chunk states exist;
        # q-side folds + intra-chunk attention fully overlap the AllGather
        nc.sync.dma_start(out=st_d[:], in_=stot)
        nc.gpsimd.collective_compute("AllGather", ALU.bypass,
                                     replica_groups=RG_PAIR,
                                     ins=[st_d[:]], outs=[stg_d[:]])
        for lc in range(2):
            lsl = slice(lc * 512, (lc + 1) * 512)
            for nt, nsz, ofs in ((2, 128, 192), (3, 64, 320)):
                qk_fold(nt, nsz, lc, lsl, ofs)
        for nt, nsz in ((2, 128), (3, 64)):
            elu1(nt, nsz, slice(0, LLOC))
        for c in range(NLT):
            csl = slice(c * 128, (c + 1) * 128)
            stm = attnp.tile([128, HL, 128], BF16, tag="stm", bufs=4)
            pst2 = psp.tile([128, 2, 128], F32, tag="tr")
            for j, h in enumerate((0, 2)):       # base-0 heads share a psum
                kt, kb = K_HEAD[h]
                qt, qb = Q_HEAD[h]
                nc.tensor.matmul(pst2[:, j, :],
                                 qkf[kb:kb + 64, kt, csl],
                                 qkf[qb:qb + 64, qt, csl],
                                 start=True, stop=True)
            nc.vector.tensor_mul(out=stm[:, 0:2, :], in0=pst2,
                                 in1=mask3[:, 0:2, :])
            pst = psp.tile([128, 128], F32, tag="tr")
            nc.tensor.matmul(pst, qkf[64:128, 0, csl], qkf[64:128, 2, csl],
                             start=True, stop=True)
            nc.vector.tensor_mul(out=stm[:, 2, :], in0=pst, in1=mask3[:, 2, :])
            po = psp.tile([128, HL, 65], F32, tag="out")
            for j, h in enumerate((0, 2, 1)):
                nc.tensor.matmul(po[:, h, :], stm[:, j, :], vaug[c][:, h, :],
                                 start=True, stop=True)
            nc.scalar.activation(out=attn_i[c], in_=po, func=AF.Copy)

        # ------- local prefix states (independent of the AllGather) -------
        sacc = big.tile([128, 2, 65], F32)
        nc.vector.memset(sacc, 0.0)
        spre = big.tile([128, NLT, 2, 65], BF16)
        for c in range(1, NLT):
            nc.vector.tensor_add(out=sacc, in0=sacc, in1=sloc[:, c - 1])
            nc.vector.tensor_copy(out=spre[:, c], in_=sacc)

        # ------------- inter-chunk attention + projection + RS -------------
        attnTA = big.tile([128, LLOC], BF16)
        attnTB = big.tile([65, LLOC], BF16)
        nc.vector.memset(attnTB[64:65, :], 1.0)
        part_d = dramp.tile([LLOC, D], F32)
        rs_d = dramp.tile([256, D], F32)
        attnr = [attnp.tile([128, DLOC], BF16, name=f"attnr{c}", tag="attnr",
                            bufs=8) for c in range(NLT)]
        # local-prefix contribution: runs inside the AllGather window
        for c in range(1, NLT):
            csl = slice(c * 128, (c + 1) * 128)
            for h in range(HL):
                qt, qb = Q_HEAD[h]
                kb = K_HEAD[h][1]
                sprh = spre[kb:kb + 64, c, 0, :] if h < 2 else spre[0:64, c, 1, :]
                po2 = psp.tile([128, 65], F32, tag="out")
                nc.tensor.matmul(po2, qkf[qb:qb + 64, qt, csl], sprh,
                                 start=True, stop=True)
                nc.vector.tensor_add(out=attn_i[c][:, h, :],
                                     in0=attn_i[c][:, h, :], in1=po2)
        # peer-state correction + epilogue (post-AllGather)
        sgsb = big.tile([128, 2, 65], BF16)
        nc.sync.dma_start(out=sgsb, in_=stg_d[0])
        sinit = big.tile([128, 2, 65], BF16)
        nc.vector.tensor_scalar_mul(out=sinit, in0=sgsb, scalar1=flag_t)
        for c in range(NLT):
            csl = slice(c * 128, (c + 1) * 128)
            for h in range(HL):
                kt, kb = K_HEAD[h]
                qt, qb = Q_HEAD[h]
                sih = sinit[kb:kb + 64, 0, :] if h < 2 else sinit[0:64, 1, :]
                po2 = psp.tile([128, 65], F32, tag="out")
                nc.tensor.matmul(po2, qkf[qb:qb + 64, qt, csl], sih,
                                 start=True, stop=True)
                nc.vector.tensor_add(out=attn_i[c][:, h, :],
                                     in0=attn_i[c][:, h, :], in1=po2)
            den = attnp.tile([128, HL], F32, tag="den", bufs=3)
            nc.gpsimd.tensor_scalar_add(out=den, in0=attn_i[c][:, :, 64],
                                        scalar1=DEN_EPS)
            nc.vector.reciprocal(out=den, in_=den)
            nc.gpsimd.tensor_mul(
                out=attnr[c].rearrange("p (h e) -> p h e", h=HL),
                in0=attn_i[c][:, :, 0:64], in1=_fbcast(den[:, :, None], 64))
            tp = psp.tile([128, 512], BF16, tag="tr")
            nc.tensor.transpose(tp[:, 0:128], attnr[c][:, 0:128], identb)
            nc.tensor.transpose(tp[0:64, 128:256], attnr[c][:, 128:192], identb)
            nc.vector.tensor_copy(out=attnTA[:, csl], in_=tp[:, 0:128])
            nc.scalar.activation(out=attnTB[0:64, csl], in_=tp[0:64, 128:256],
                                 func=AF.Copy)
            for ns in range(2):
                nsl = slice(ns * 384, (ns + 1) * 384)
                pp = psp.tile([128, 512], F32, tag="mm", bufs=3)
                nc.tensor.matmul(pp[:, 0:384], attnTA[:, csl], wpra[:, nsl],
                                 start=True, stop=False)
                nc.tensor.matmul(pp[:, 0:384], attnTB[:, csl], wprb[:, nsl],
                                 start=False, stop=True)
                pout = work.tile([128, 384], F32, tag="pout")
                nc.vector.tensor_copy(out=pout, in_=pp[:, 0:384])
                nc.sync.dma_start(out=part_d[csl, nsl], in_=pout)
        nc.gpsimd.collective_compute(
            "ReduceScatter", ALU.add, replica_groups=RG_QUAD,
            ins=[part_d[:]], outs=[rs_d[:]])
        nc.sync.dma_start(out=out_p[:], in_=rs_d[:])

    nc.finalize()
    return nc


_NC_CACHE = None


def _get_nc():
    global _NC_CACHE
    if _NC_CACHE is None:
        import os
        _NC_CACHE = build(stage=int(os.environ.get("KSTAGE", "99")))
    return _NC_CACHE


def _prep_in_maps(inputs):
    x = np.ascontiguousarray(np.asarray(inputs["x"], np.float32))[0]
    W_qkv = np.asarray(inputs["W_qkv"], np.float32)
    b_qkv = np.asarray(inputs["b_qkv"], np.float32)
    W_sem = np.asarray(inputs["W_sem"], np.float32)
    b_sem = np.asarray(inputs["b_sem"], np.float32)
    W_ctx = np.asarray(inputs["W_ctx"], np.float32)
    b_ctx = np.asarray(inputs["b_ctx"], np.float32)
    W_proj = np.asarray(inputs["W_proj"], np.float32)
    b_proj = np.asarray(inputs["b_proj"], np.float32)
    ln_g = np.asarray(inputs["ln_g"], np.float32)
    ln_b = np.asarray(inputs["ln_b"], np.float32)

    Wg = ln_g[:, None] * W_qkv
    bias2 = ln_b @ W_qkv + b_qkv
    assert not np.any(bias2), "nonzero qkv bias not supported by this kernel"

    import ml_dtypes
    c = lambda a: np.ascontiguousarray(a, dtype=np.float32)
    cb = lambda a: np.ascontiguousarray(np.asarray(a, np.float32).astype(ml_dtypes.bfloat16))
    in_maps = []
    for core in range(8):
        sg, hg = core // 4, core % 4
        idx = slice(DLOC * hg, DLOC * hg + DLOC)
        Wk_c = Wg[:, 768:1536][:, idx]
        Wq_c = Wg[:, 0:768][:, idx]
        Z = np.zeros((D, 64), np.float32)
        w_qk = np.concatenate([Wk_c, Wq_c], 1)          # [768, 384], no pads
        w_qk_pad = np.concatenate([Wk_c, Z, Wq_c, Z], 1)  # ncs keeps tile layout
        Wv_c = Wg[:, 1536:][:, idx]
        in_maps.append({
            "x": c(x[LLOC * sg:LLOC * sg + LLOC]),
            "w_qk": cb(w_qk),
            "w_g4": cb(np.concatenate([
                W_sem[:, 0:768][:, idx], W_ctx[:, 0:768][:, idx],
                W_sem[:, 768:][:, idx] - W_ctx[:, 768:][:, idx], Wv_c], 1)),
            "w_pra": cb(W_proj[idx, :][0:128]),
            "w_prb": cb(np.concatenate([W_proj[idx, :][128:192],
                                        b_proj[None, :] / HG], 0)),
            "ncs_qk": c(-w_qk_pad.sum(0)),
            "cs_v": c(Wv_c.sum(0)),
            "ph_b": c(np.pi / 2 + (b_sem[768:][idx] - b_ctx[768:][idx])),
            "bs_a": c(b_sem[:768][idx]),
            "bc_a": c(b_ctx[:768][idx]),
            "flag": c(np.array([float(sg)])),
        })
    return in_maps


def _run(inputs, trace=False):
    nc = _get_nc()
    in_maps = _prep_in_maps(inputs)
    res = run_bass_kernel_spmd(nc, in_maps, core_ids=list(range(8)), trace=trace)
    out = np.zeros((L, D), np.float32)
    for core in range(8):
        sg, hg = core // 4, core % 4
        o = res.results[core]["out"]          # [256, D]
        r0 = LLOC * sg + 256 * hg
        out[r0:r0 + 256] = o
    return out[None], res


def kernel(**inputs):
    out, _ = _run(inputs, trace=False)
    return out
